# revision 25
# baseline (speedup 1.0000x reference)
"""Trainium2 Bass kernel for nn_Bert (VOCAB=9, D=4, S=16384) on 8 NeuronCores.

Key identity: with a tiny vocabulary (9) and tiny width (4), every row of the
reference output depends only on the token id x[s] and the *global* histogram
c_v of x:

    T = emb @ proj_w.T + proj_b                       (9,4)  per-token h1
    G = T @ T.T                                       (9,9)  symmetric score table
    attn_out(a) = sum_v c_v e^{G[a,v]} T[v] / sum_v c_v e^{G[a,v]}
    F = softmax(relu(attn_out) @ M2.T + b2)           (9,9)  final per-token table
        where M2 = prj_w @ forw_w, b2 = prj_w @ forw_b + prj_b
        (the two affine layers after the relu compose into one)
    out[s] = F[x[s]]

Device schedule per core (sequence row-sharded, 2048 positions/core):
  - table matmuls (independent of the histogram) are emitted first: the PE
    executes its queue in order, so they must precede the c matmul
  - histogram of the full x: 9 WAW-free fused is_equal+accum DVE ops, then
    one partition-reduce matmul
  - 9x9 table math; the relu/bias chain is collapsed via an augmented
    [T | 1] operand so one matmul yields both Sh^T and Z (relu passes Z
    through), and the bias row b2 is folded via Z: P = RTa^T @ [M2.T; b2],
    logits = P * (1/Z); softmax row sums on DVE; F is split hi/lo into
    bf16 without materializing fp32 F (fused TTSS op)
  - final gather as a one-hot matmul in bf16: the four 512-column chunks
    run CONCURRENTLY in four 32-partition strips of the PE array
    (tile_position col-tiling), each into its own PSUM bank; eviction
    copies alternate ACT/DVE and the four output DMAs alternate across
    both HWDGE rings
"""

import os
from contextlib import ExitStack

import ml_dtypes
import numpy as np

import concourse.tile as tile
from concourse import bacc, mybir
from concourse._compat import get_trn_type
from concourse.bass_utils import run_bass_kernel_spmd

VOCAB = 9
D = 4
S = 16384
NCORES = 8
SLICE = S // NCORES  # 2048
NCHUNK = 4           # 512-column matmul chunks of the per-core slice
CHUNK = SLICE // NCHUNK

F32 = mybir.dt.float32
BF16 = mybir.dt.bfloat16

# Packed constants layout, one [128, 33] f32 tensor:
#   col 0      : ones (rows 0..127)
#   cols 1:5   : A  = [proj_w.T; proj_b]  rows 0..4   (K=5 augmented proj)
#   cols 5:14  : B  = [emb.T; ones(9)]   rows 0..4
#   cols 14:23 : D2 = [M2.T; b2]         rows 0..4    (folded forw+classifier)
#   col 23     : iota9 (rows 0..8 = 0..8)
#   cols 24:33 : spare
NCONST = 33

LAST_RESULTS = None  # BassKernelResults of the most recent run (for test.py)


def build_nc():
    nc = bacc.Bacc(
        get_trn_type() or "TRN2",
        target_bir_lowering=False,
        debug=False,
        enable_asserts=False,
        num_devices=NCORES,
    )
    xall = nc.dram_tensor("xall", [128, 128], BF16, kind="ExternalInput")
    xqrep = nc.dram_tensor("xqrep", [VOCAB, SLICE], BF16, kind="ExternalInput")
    consts = nc.dram_tensor("consts", [128, NCONST], F32, kind="ExternalInput")
    outT = nc.dram_tensor(
        "outT", [NCHUNK, VOCAB, CHUNK], F32, kind="ExternalOutput"
    )

    with tile.TileContext(nc) as tc:
        with ExitStack() as ctx:
            _build_kernel(ctx, tc, xall.ap(), xqrep.ap(), consts.ap(), outT.ap())
    nc.compile()
    return nc


def _build_kernel(ctx, tc, xall, xqrep, consts, outT):
    nc = tc.nc
    pool = ctx.enter_context(tc.tile_pool(name="sbuf", bufs=1))
    psum = ctx.enter_context(tc.tile_pool(name="psum", bufs=4, space="PSUM"))
    psum_out = ctx.enter_context(tc.tile_pool(name="psum_out", bufs=4, space="PSUM"))

    # ---- input DMAs on three different queues so they overlap ----
    x_s = pool.tile([128, 128], BF16)
    nc.sync.dma_start(x_s[:], xall)
    const_s = pool.tile([128, NCONST], F32)
    nc.scalar.dma_start(const_s[:], consts)
    xq_s = pool.tile([VOCAB, SLICE], BF16)
    nc.gpsimd.dma_start(xq_s[:], xqrep)

    ones128 = const_s[0:128, 0:1]
    ones9 = const_s[0:VOCAB, 0:1]
    A_s = const_s[0:5, 1:5]
    B_s = const_s[0:5, 5:14]
    D2_s = const_s[0:5, 14:23]
    iota9 = const_s[0:VOCAB, 23:24]

    # ---- per-token tables first: they are independent of the histogram, and
    # the PE executes in order, so they must precede c_mm in the queue ----
    # T_T[d, a] (bias folded via the augmented K=5 contraction), T[a, d]
    TT_ps = psum.tile([D, VOCAB], F32, tag="small")
    nc.tensor.matmul(TT_ps[:], A_s, B_s)
    T_ps = psum.tile([VOCAB, D], F32, tag="small")
    nc.tensor.matmul(T_ps[:], B_s, A_s)
    TT_s = pool.tile([D, VOCAB], F32)
    nc.scalar.copy(TT_s[:], TT_ps[:])
    # T1 = [T | 1]: the ones column makes the ShT matmul also produce Z
    T1_s = pool.tile([VOCAB, D + 1], F32)
    nc.gpsimd.memset(T1_s[:], 1.0)
    nc.scalar.copy(T1_s[:, 0:D], T_ps[:])

    # G[a, v] = T[a] . T[v]  (symmetric)
    G_ps = psum.tile([VOCAB, VOCAB], F32, tag="small")
    nc.tensor.matmul(G_ps[:], TT_s[:], TT_s[:])
    E_s = pool.tile([VOCAB, VOCAB], F32)
    nc.scalar.activation(E_s[:], G_ps[:], mybir.ActivationFunctionType.Exp)

    # ---- histogram of the full x: H[p, v] = sum_f (x[p,f] == v) ----
    # nine WAW-free accum-fused compares (separate output slices)
    ohb = pool.tile([128, VOCAB, 128], BF16)
    H = pool.tile([128, VOCAB], F32)
    for v in range(VOCAB):
        last_cmp = nc.vector.tensor_scalar(
            out=ohb[:, v, :],
            in0=x_s[:],
            scalar1=float(v),
            scalar2=None,
            op0=mybir.AluOpType.is_equal,
            op1=mybir.AluOpType.add,
            accum_out=H[:, v : v + 1],
        )
    c_ps = psum.tile([VOCAB, 1], F32, tag="small")
    nc.tensor.matmul(c_ps[:], H[:], ones128)  # c[v] = sum_p H[p, v]

    # W[v, a] = c_v * exp(G[v, a])
    W_s = pool.tile([VOCAB, VOCAB], F32)
    nc.vector.tensor_scalar(
        out=W_s[:],
        in0=E_s[:],
        scalar1=c_ps[:],
        scalar2=None,
        op0=mybir.AluOpType.mult,
    )

    # One matmul gives rows 0-3 = Sh^T[d, a] and row 4 = Z[a]; relu keeps
    # Z (positive) unchanged, so a single Relu yields the augmented operand.
    ShTa_ps = psum.tile([D + 1, VOCAB], F32, tag="small")
    nc.tensor.matmul(ShTa_ps[:], T1_s[:], W_s[:])
    RTa_s = pool.tile([D + 1, VOCAB], F32)
    nc.scalar.activation(RTa_s[:], ShTa_ps[:], mybir.ActivationFunctionType.Relu)

    # Zr[a] = 1/Z[a] (off the critical path; used as the exp scale)
    Z_ps = psum.tile([VOCAB, 1], F32, tag="small")
    nc.tensor.matmul(Z_ps[:], W_s[:], ones9)
    Zr_s = pool.tile([VOCAB, 1], F32)
    zr_inst = nc.vector.reciprocal(Zr_s[:], Z_ps[:])

    # ---- one-hot for the final gather ----
    # ohT[v, s] = (xq[s] == v), bf16 (exact 0/1); pinned after Zr in the DVE
    # queue so it does not delay the critical softmax chain
    oh_s = pool.tile([VOCAB, SLICE], BF16)
    oh_inst = nc.vector.tensor_scalar(
        out=oh_s[:],
        in0=xq_s[:],
        scalar1=iota9,
        scalar2=None,
        op0=mybir.AluOpType.is_equal,
    )
    tile.add_dep_helper(
        oh_inst.ins, zr_inst.ins, sync=False, reason="oh after Zr on DVE"
    )

    # P[a, j] = sum_d relu(ShT)[d, a] M2[j, d] + Z[a] b2[j]
    # => logits[a, j] = P[a, j] * Zr[a]
    P_ps = psum.tile([VOCAB, VOCAB], F32, tag="small")
    nc.tensor.matmul(P_ps[:], RTa_s[:], D2_s)

    # F[a, j] = softmax_j(logits[a, :])  (row sum fused into the exp)
    expL_s = pool.tile([VOCAB, VOCAB], F32)
    nc.scalar.activation(
        expL_s[:], P_ps[:], mybir.ActivationFunctionType.Exp, scale=Zr_s[:]
    )
    Ssum_s = pool.tile([VOCAB, 1], F32)
    nc.vector.tensor_reduce(
        Ssum_s[:], expL_s[:], axis=mybir.AxisListType.X, op=mybir.AluOpType.add
    )
    Sr_s = pool.tile([VOCAB, 1], F32)
    nc.vector.reciprocal(Sr_s[:], Ssum_s[:])

    # exact bf16 hi/lo split of F = expL*Sr, without materializing fp32 F:
    #   Fhi = bf16(expL*Sr);  Flo = (Fhi - expL*Sr)*(-1) via the fused TTSS op
    Fhi_s = pool.tile([VOCAB, VOCAB], BF16)
    nc.vector.tensor_scalar(
        out=Fhi_s[:],
        in0=expL_s[:],
        scalar1=Sr_s[:],
        scalar2=None,
        op0=mybir.AluOpType.mult,
    )
    Flo_s = pool.tile([VOCAB, VOCAB], BF16)
    nc.vector.ln_bwd_dx(
        out=Flo_s[:],
        dy=Fhi_s[:],
        x_hat=expL_s[:],
        mean_dyx=Sr_s[:],
        mean_dy=0.0,
        scale=-1.0,
    )

    # ---- final gather: outT[j, s] = sum_v F[v, j] * (xq[s] == v) ----
    # column-tiled: the four 512-column chunks run CONCURRENTLY in four
    # 32-partition strips of the PE array, each into ITS OWN psum bank so
    # the post-copies don't serialize on one bank
    o_pss = [
        psum_out.tile([128, CHUNK], F32, tag="obank", name=f"o_ps{i}")
        for i in range(NCHUNK)
    ]
    for cidx in range(NCHUNK):
        sl = slice(cidx * CHUNK, (cidx + 1) * CHUNK)
        nc.tensor.matmul(
            o_pss[cidx][32 * cidx : 32 * cidx + VOCAB, :],
            Fhi_s[:],
            oh_s[:, sl],
            start=True,
            stop=False,
            tile_position=(0, 32 * cidx),
            skip_group_check=True,
        )
    for cidx in range(NCHUNK):
        sl = slice(cidx * CHUNK, (cidx + 1) * CHUNK)
        nc.tensor.matmul(
            o_pss[cidx][32 * cidx : 32 * cidx + VOCAB, :],
            Flo_s[:],
            oh_s[:, sl],
            start=False,
            stop=True,
            tile_position=(0, 32 * cidx),
            skip_group_check=True,
        )
    outT_s = pool.tile([128, CHUNK], F32)
    dma_engs = [nc.sync, nc.scalar, nc.sync, nc.scalar]
    for cidx in range(NCHUNK):
        rows = slice(32 * cidx, 32 * cidx + VOCAB)
        if cidx % 2 == 0:
            nc.scalar.copy(outT_s[rows, :], o_pss[cidx][rows, :])
        else:
            nc.vector.tensor_copy(outT_s[rows, :], o_pss[cidx][rows, :])
        dma_engs[cidx].dma_start(outT[cidx], outT_s[rows, :])


def host_prep(x, emb, proj_w, proj_b, forw_w, forw_b, prj_w, prj_b):
    """Pack weights/constants and per-core sharded inputs."""
    f32 = np.float32
    x = np.asarray(x).reshape(-1).astype(np.int64)
    assert x.shape == (S,)
    emb = np.asarray(emb, f32)
    proj_w = np.asarray(proj_w, f32)
    proj_b = np.asarray(proj_b, f32)
    forw_w = np.asarray(forw_w, f32)
    forw_b = np.asarray(forw_b, f32)
    prj_w = np.asarray(prj_w, f32)
    prj_b = np.asarray(prj_b, f32)

    M2 = (prj_w @ forw_w).astype(f32)          # (9, 4)
    b2 = (prj_w @ forw_b + prj_b).astype(f32)  # (9,)

    consts = np.zeros((128, NCONST), f32)
    consts[:, 0] = 1.0
    consts[0:4, 1:5] = proj_w.T
    consts[4, 1:5] = proj_b
    consts[0:4, 5:14] = emb.T
    consts[4, 5:14] = 1.0
    consts[0:4, 14:23] = M2.T
    consts[4, 14:23] = b2
    consts[0:VOCAB, 23] = np.arange(VOCAB, dtype=f32)

    xall = x.reshape(128, 128).astype(ml_dtypes.bfloat16)
    in_maps = []
    for i in range(NCORES):
        xq = x[i * SLICE : (i + 1) * SLICE].astype(ml_dtypes.bfloat16)
        in_maps.append(
            {
                "xall": xall,
                "consts": consts,
                "xqrep": np.ascontiguousarray(
                    np.broadcast_to(xq[None, :], (VOCAB, SLICE))
                ),
            }
        )
    return in_maps


_NC_CACHE = None


def kernel(x, emb, proj_w, proj_b, forw_w, forw_b, prj_w, prj_b):
    global _NC_CACHE, LAST_RESULTS
    if _NC_CACHE is None:
        _NC_CACHE = build_nc()
    nc = _NC_CACHE
    in_maps = host_prep(x, emb, proj_w, proj_b, forw_w, forw_b, prj_w, prj_b)
    trace = bool(os.environ.get("BASS_TRACE"))
    res = run_bass_kernel_spmd(nc, in_maps, list(range(NCORES)), trace=trace)
    LAST_RESULTS = res
    out = np.empty((S, VOCAB), np.float32)
    for i in range(NCORES):
        arr = res.results[i]["outT"]  # (NCHUNK, VOCAB, CHUNK)
        out[i * SLICE : (i + 1) * SLICE, :] = arr.transpose(0, 2, 1).reshape(
            SLICE, VOCAB
        )
    return out
